# revision 40
# baseline (speedup 1.0000x reference)
"""Multi-head attention (B=4, S=2048, D=1024, H=16) on 8 Trainium2 NeuronCores.

Sharding: 4-way data-parallel over batch x 2-way tensor-parallel over heads
(Megatron-style).  Core c handles batch c//2 and head-group c%2 (8 of 16
heads).  Each core computes qkv for its 512 q/k/v channels, attention for its
8 heads, and a row-parallel partial projection [S, D].  The host sums the two
partial outputs per batch and adds b_proj.

v2 strategy (all-bf16, ScalarE-exp-bound pipeline):
  - Host pre-transposes x to x^T and casts all operands to bf16 (rel-err
    budget 2e-2 leaves ~30x margin; bf16 matmul runs at full PE rate and
    needs no on-device f32r rounding copies or PE transposes).
  - Q^T/K^T computed channel-major [ch, s] (w stationary, x^T moving);
    V natural [s, ch] (x^T chunks stationary, w_v moving) with a ones
    column appended per head for softmax row-sums.
  - Attention per (head-pair, 512-q block): the two heads of a ch-chunk sit
    in partition halves 0:64 / 64:128, so their QK matmuls use PE row groups
    (0,0)/(64,0) and overlap on HW.  One exp per kj covers both heads
    (N=1024 from PSUM).  PV is emitted one kj behind QK/exp so the PE FIFO
    never head-blocks the ScalarE exp stream (the kernel is exp-bound).
  - attn accumulators (with row-sums on partition 64 via the ones column)
    are evicted PSUM->SBUF by GpSimd immediately so the 2 attn banks
    recycle; softmax normalization (approx-reciprocal + partition-broadcast
    + mul) runs off the critical path on DVE/GpSimd; odd heads reach
    partitions 64:128 of the proj lhsT via a small SBUF->SBUF DMA (compute
    engines cannot shift partitions).
  - Q for later q-blocks and the projection of the previous q-block are
    interleaved into the attention stream as fillers under the exp window.
"""

import sys
from contextlib import ExitStack

for _p in ("/opt/trn_rl_repo", "/root/.axon_site/_ro/trn_rl_repo"):
    if _p not in sys.path:
        sys.path.insert(0, _p)

import numpy as np

import concourse.bass as bass  # noqa: F401
import concourse.mybir as mybir
import concourse.tile as tile
from concourse import bacc
from concourse.bass_utils import run_bass_kernel_spmd
from concourse.masks import make_identity

F32 = mybir.dt.float32
BF16 = mybir.dt.bfloat16
EXP = mybir.ActivationFunctionType.Exp
BF16_NP = mybir.dt.np(BF16)

N_CORES = 8
FULL_B, FULL_S, FULL_D, FULL_H = 4, 2048, 1024, 16
HEAD_DIM = 64


def build_core_program(S=FULL_S, D=FULL_D, HL=FULL_H // 2, hd=HEAD_DIM,
                       dve_kj=(3, 7, 11, 15), v_in_attn=True, tail_fast=True,
                       attn_reps=1):
    CH = HL * hd            # local q (= k = v) channels (512)
    DC = D // 128           # d-chunks (qkv contraction)
    CCQ = CH // 128         # 128-ch chunks of q/k = head pairs (4)
    SC = S // 128           # 128-row s-chunks (kj)
    QBS = 512               # q block size in attention
    QB = S // QBS
    scale = float(hd) ** -0.5
    # kj steps per block whose exp runs on DVE (Schraudolph bitcast
    # approximation in bf16 bit-space, ~1.6% rms on those keys) instead of
    # ScalarE, spreading the exp bottleneck over two engines.
    # exp(s*scale) ~= bitcast_bf16(int16(s * SCH_A + SCH_B)); the DVE
    # f32->i16 conversion truncates, C=6.92 centers the relative error
    # (CoreSim-measured, probe_dve16.py).
    DVE_KJ = tuple(dve_kj)
    SCH_A = 128.0 / float(np.log(2.0)) * scale
    SCH_B = 127.0 * 128.0 - 6.92

    nc = bacc.Bacc("TRN2", target_bir_lowering=False, debug=False,
                   num_devices=N_CORES)

    xT_ap = nc.dram_tensor("xT", [D, S], BF16, kind="ExternalInput").ap()
    wqk_ap = nc.dram_tensor("w_qk", [D, 2 * CH], BF16, kind="ExternalInput").ap()
    wv_ap = nc.dram_tensor("w_v", [D, CH], BF16, kind="ExternalInput").ap()
    wp_ap = nc.dram_tensor("w_proj", [CH, D], BF16, kind="ExternalInput").ap()
    bqk_ap = nc.dram_tensor("b_qk", [2 * CH], F32, kind="ExternalInput").ap()
    bv_ap = nc.dram_tensor("b_v", [CH], F32, kind="ExternalInput").ap()
    out_ap = nc.dram_tensor("out", [S, D], F32, kind="ExternalOutput").ap()

    with tile.TileContext(nc) as tc, ExitStack() as es:
        constp = es.enter_context(tc.tile_pool(name="const", bufs=1))
        actp = es.enter_context(tc.tile_pool(name="acts", bufs=1))
        workp = es.enter_context(tc.tile_pool(name="work", bufs=1, side="right"))
        ps_gen = es.enter_context(tc.tile_pool(name="psg", bufs=1, space="PSUM"))
        ps_att = es.enter_context(tc.tile_pool(name="psa", bufs=1, space="PSUM"))

        # ---- constants / weights / x^T loads (all bf16, no conversion) ----
        bias_qk = constp.tile([128, 2 * CCQ], F32)
        nc.sync.dma_start(bias_qk[:], bqk_ap.rearrange("(c p) -> p c", p=128))
        bv_row = constp.tile([1, CH], F32)
        nc.sync.dma_start(bv_row[:], bv_ap.rearrange("(a b) -> a b", a=1))
        bv_bc = constp.tile([128, CH], F32)
        nc.gpsimd.partition_broadcast(bv_bc[:], bv_row[0:1, :])
        warm = constp.tile([1, 16], F32)
        nc.vector.memset(warm[:], 0.0)
        # pull the exp table-load off the critical path
        nc.scalar.activation(warm[:], warm[:], EXP)
        if tail_fast:
            ones65_f = constp.tile([65, 64], F32)
            nc.vector.memset(ones65_f[:], 1.0)
            ones65 = constp.tile([65, 64], mybir.dt.float32r)
            nc.vector.tensor_copy(ones65[:], ones65_f[:])
            ident64 = constp.tile([64, 64], BF16)
            make_identity(nc, ident64[:])

        wqk_r = constp.tile([128, DC, 2 * CH], BF16)
        wv_r = constp.tile([128, DC, CH], BF16)
        xT = constp.tile([128, DC, S], BF16)
        wp_r = constp.tile([128, CCQ, D], BF16)
        # wqk first, then x^T by s-block so the first K chain (which needs all
        # d-chunks but only s-block 0) can start ~10us in; wv/wp stream later.
        for dc in range(DC):
            nc.sync.dma_start(wqk_r[:, dc, :], wqk_ap[dc * 128:(dc + 1) * 128, :])
        for sb in range(S // 512):
            for dc in range(DC):
                nc.sync.dma_start(
                    xT[:, dc, sb * 512:(sb + 1) * 512],
                    xT_ap[dc * 128:(dc + 1) * 128, sb * 512:(sb + 1) * 512])
        for dc in range(DC):
            nc.sync.dma_start(wv_r[:, dc, :], wv_ap[dc * 128:(dc + 1) * 128, :])
        for cc in range(CCQ):
            nc.sync.dma_start(wp_r[:, cc, :], wp_ap[cc * 128:(cc + 1) * 128, :])

        # ---- persistent activations ----
        qT = actp.tile([128, CCQ, S], BF16)           # [ch, s]
        kT = actp.tile([128, CCQ, S], BF16)
        vp = actp.tile([128, SC, HL, hd + 2], BF16)   # [s|kj, head, V|1|pad]
        nc.vector.memset(vp[:, :, :, hd], 1.0)
        attn_r = actp.tile([128, CCQ, S], BF16)       # normalized attn^T

        # ---- generation chain-groups (lead-in + fillers) ----
        def gen_qk(cc, sb, dst):
            """one [128ch, 512s] chunk of Q^T (dst=0) or K^T (dst=1)."""
            gp = ps_gen.tile([128, 512], F32, tag="gen", bufs=2)
            for dc in range(DC):
                nc.tensor.matmul(gp[:],
                                 wqk_r[:, dc, dst * CH + cc * 128:
                                       dst * CH + (cc + 1) * 128],
                                 xT[:, dc, sb * 512:(sb + 1) * 512],
                                 start=(dc == 0), stop=(dc == DC - 1))
            tgt = qT if dst == 0 else kT
            nc.vector.tensor_scalar_add(
                tgt[:, cc, sb * 512:(sb + 1) * 512], gp[:],
                bias_qk[:, dst * CCQ + cc:dst * CCQ + cc + 1])

        def gen_v(sc):
            """one [128s, 512ch] chunk of V (natural), bias added."""
            gp = ps_gen.tile([128, CH], F32, tag="gen", bufs=2)
            for dc in range(DC):
                nc.tensor.matmul(gp[:],
                                 xT[:, dc, sc * 128:(sc + 1) * 128],
                                 wv_r[:, dc, :],
                                 start=(dc == 0), stop=(dc == DC - 1))
            nc.vector.tensor_add(
                vp[:, sc, :, 0:hd],
                gp[:].rearrange("p (h e) -> p h e", e=hd),
                bv_bc[:].rearrange("p (h e) -> p h e", e=hd))


        def gen_proj(qb, sc_i):
            """projection for 128 q rows of block qb (both 512-d halves)."""
            sc_g = qb * (QBS // 128) + sc_i
            for dh in range(2):
                pp = ps_gen.tile([128, 512], F32, tag="gen", bufs=2)
                for cc in range(CCQ):
                    nc.tensor.matmul(pp[:],
                                     attn_r[:, cc, sc_g * 128:(sc_g + 1) * 128],
                                     wp_r[:, cc, dh * 512:(dh + 1) * 512],
                                     start=(cc == 0), stop=(cc == CCQ - 1))
                osb = workp.tile([128, 512], F32, tag="osb", bufs=3)
                nc.vector.tensor_copy(osb[:], pp[:])
                nc.sync.dma_start(
                    out_ap[sc_g * 128:(sc_g + 1) * 128,
                           dh * 512:(dh + 1) * 512], osb[:])

        # ---- lead-in: K (all s), Q(qb0).  V is generated just-in-time
        # inside the first attention block (one chunk per kj step, emitted
        # ahead of the PV that consumes it) so exp starts ~45us earlier. ----
        for sb in range(S // 512):
            for cc in range(CCQ):
                gen_qk(cc, sb, 1)
        for cc in range(CCQ):
            gen_qk(cc, 0, 0)
        if v_in_attn:
            gen_v(0)
        else:
            for sc in range(SC):
                gen_v(sc)

        # filler queue, consumed inside the attention stream
        fillers = []
        for sb in range(1, QB):
            for cc in range(CCQ):
                fillers.append((gen_qk, (cc, sb, 0)))
        f_i = [0]

        def pump():
            if f_i[0] < len(fillers):
                fn, args = fillers[f_i[0]]
                f_i[0] += 1
                fn(*args)

        def pv_pair(pt_t, kj, attnA, attnB, cc):
            nc.tensor.matmul(attnA[:], vp[:, kj, 2 * cc, 0:hd + 1],
                             pt_t[:, 0, :],
                             start=(kj == 0), stop=(kj == SC - 1))
            nc.tensor.matmul(attnB[:], vp[:, kj, 2 * cc + 1, 0:hd + 1],
                             pt_t[:, 1, :],
                             start=(kj == 0), stop=(kj == SC - 1))

        def emit_norm(cc, qb, attn_sbA, attn_sbB):
            """softmax-normalize both heads of a finished (pair, qb) block.
            Row sums live on partition 64 of the evicted accumulators; a tiny
            SBUF->SBUF DMA moves them to partition 0 (engines cannot)."""
            sums0 = workp.tile([1, 2 * QBS], F32, tag="sums0", bufs=2)
            nc.sync.dma_start(sums0[0:1, 0:QBS], attn_sbA[64:65, :])
            nc.sync.dma_start(sums0[0:1, QBS:2 * QBS], attn_sbB[64:65, :])
            rec = workp.tile([1, 2 * QBS], F32, tag="rec", bufs=2)
            nc.vector.reciprocal_approx_fast(rec[:], sums0[:])
            rbc = workp.tile([64, 2 * QBS], F32, tag="rbc", bufs=2)
            nc.gpsimd.partition_broadcast(rbc[:], rec[0:1, :])
            nc.vector.tensor_mul(
                attn_r[0:64, cc, qb * QBS:(qb + 1) * QBS],
                attn_sbA[0:64, :], rbc[:, 0:QBS])
            stg = workp.tile([64, QBS], BF16, tag="stg", bufs=2)
            nc.vector.tensor_mul(stg[:], attn_sbB[0:64, :],
                                 rbc[:, QBS:2 * QBS])
            nc.sync.dma_start(
                attn_r[64:128, cc, qb * QBS:(qb + 1) * QBS], stg[:])

        def emit_norm_fast(cc, qb, attnA, attnB):
            """tail variant: normalize straight from PSUM with a PE row-sum
            broadcast and a PE partition-shift for the odd head, avoiding the
            two DMA round-trips on the final critical path."""
            sums = workp.tile([65, 2 * QBS], mybir.dt.float32r,
                              tag="fsums", bufs=1)
            nc.vector.tensor_copy(sums[64:65, 0:QBS], attnA[64:65, :])
            nc.vector.tensor_copy(sums[64:65, QBS:2 * QBS], attnB[64:65, :])
            bc_t = ps_att.tile([128, 2, QBS], F32, tag="sc", bufs=2)
            nc.tensor.matmul(bc_t[0:64, 0, :], ones65[64:65, 0:64],
                             sums[64:65, 0:QBS], start=True, stop=True,
                             tile_position=(64, 0))
            nc.tensor.matmul(bc_t[0:64, 1, :], ones65[64:65, 0:64],
                             sums[64:65, QBS:2 * QBS], start=True, stop=True,
                             tile_position=(64, 0))
            rbcf = workp.tile([64, 2, QBS], F32, tag="rbcf", bufs=1)
            nc.vector.reciprocal_approx_fast(rbcf[:], bc_t[0:64, :, :])
            nc.vector.tensor_mul(
                attn_r[0:64, cc, qb * QBS:(qb + 1) * QBS],
                attnA[0:64, :], rbcf[:, 0, :])
            stg = workp.tile([64, QBS], BF16, tag="stg", bufs=2)
            nc.vector.tensor_mul(stg[:], attnB[0:64, :], rbcf[:, 1, :])
            sh_t = ps_att.tile([128, 2, QBS], F32, tag="sc", bufs=2)
            nc.tensor.matmul(sh_t[64:128, 0, :], ident64[:], stg[:],
                             start=True, stop=True, tile_position=(0, 64))
            nc.vector.tensor_copy(
                attn_r[64:128, cc, qb * QBS:(qb + 1) * QBS],
                sh_t[64:128, 0, :])

        # ---- attention (exp-bound steady state) ----
        for qb in range(QB):
            for cc in range(CCQ):
                attnA = ps_att.tile([65, QBS], F32, tag="attnA", bufs=1)
                attnB = ps_att.tile([65, QBS], F32, tag="attnB", bufs=1)
                first_blk = (qb == 0 and cc == 0) and v_in_attn
                prev = None
                # attn_reps>1 is a measurement-linearity probe (wrong math)
                for kj in [k for _ in range(attn_reps) for k in range(SC)]:
                    sc_t = ps_att.tile([128, 2, QBS], F32, tag="sc", bufs=2)
                    nc.tensor.matmul(
                        sc_t[:, 0, :],
                        kT[0:64, cc, kj * 128:(kj + 1) * 128],
                        qT[0:64, cc, qb * QBS:(qb + 1) * QBS],
                        start=True, stop=True, tile_position=(0, 0))
                    nc.tensor.matmul(
                        sc_t[:, 1, :],
                        kT[64:128, cc, kj * 128:(kj + 1) * 128],
                        qT[64:128, cc, qb * QBS:(qb + 1) * QBS],
                        start=True, stop=True, tile_position=(64, 0))
                    pt_t = workp.tile([128, 2, QBS], BF16, tag="pt", bufs=4)
                    if kj in DVE_KJ and not first_blk:
                        # Schraudolph exp in bf16 bit-space: DVE f32->i16
                        # convert truncates; the int16 IS the bf16 pattern.
                        nc.vector.tensor_scalar(
                            pt_t[:].bitcast(mybir.dt.int16), sc_t[:],
                            SCH_A, SCH_B,
                            mybir.AluOpType.mult, mybir.AluOpType.add)
                    else:
                        nc.scalar.activation(pt_t[:], sc_t[:], EXP, scale=scale)
                    if first_blk and kj < SC - 1:
                        gen_v(kj + 1)  # stays one step ahead of the PVs
                    if prev is not None:
                        pv_pair(prev[1], prev[0], attnA, attnB, cc)
                    if not first_blk and kj % 4 == 1:
                        pump()
                    prev = (kj, pt_t)
                pv_pair(prev[1], prev[0], attnA, attnB, cc)
                if tail_fast and qb == QB - 1 and cc == CCQ - 1:
                    emit_norm_fast(cc, qb, attnA, attnB)
                else:
                    # evict accumulators so the 2 attn banks recycle fast
                    attn_sbA = workp.tile([65, QBS], F32, tag="asbA", bufs=2)
                    attn_sbB = workp.tile([65, QBS], F32, tag="asbB", bufs=2)
                    nc.vector.tensor_copy(attn_sbA[:], attnA[:])
                    nc.vector.tensor_copy(attn_sbB[:], attnB[:])
                    emit_norm(cc, qb, attn_sbA, attn_sbB)
            if qb > 0:
                for sc_i in range(QBS // 128):
                    fillers.append((gen_proj, (qb - 1, sc_i)))
        while f_i[0] < len(fillers):
            pump()
        for sc_i in range(QBS // 128):
            gen_proj(QB - 1, sc_i)

    nc.compile()
    return nc


def shard_inputs(x, w_qkv, b_qkv, w_proj):
    """Full inputs -> per-core input maps (host does transpose + bf16 cast).
    Core c: batch c//2, head-group c%2."""
    B, S, D = x.shape
    CH = D // 2
    in_maps = []
    for c in range(N_CORES):
        b, g = c // 2, c % 2
        sl = slice(g * CH, (g + 1) * CH)
        w_qk = np.concatenate(
            [w_qkv[:, 0 * D + g * CH:0 * D + (g + 1) * CH],
             w_qkv[:, 1 * D + g * CH:1 * D + (g + 1) * CH]],
            axis=1).astype(BF16_NP)
        w_v = np.ascontiguousarray(
            w_qkv[:, 2 * D + g * CH:2 * D + (g + 1) * CH]).astype(BF16_NP)
        b_qk = np.concatenate(
            [b_qkv[0 * D + g * CH:0 * D + (g + 1) * CH],
             b_qkv[1 * D + g * CH:1 * D + (g + 1) * CH]], axis=0)
        in_maps.append({
            "xT": np.ascontiguousarray(x[b].T).astype(BF16_NP),
            "w_qk": w_qk,
            "w_v": w_v,
            "w_proj": np.ascontiguousarray(w_proj[sl, :]).astype(BF16_NP),
            "b_qk": np.ascontiguousarray(b_qk),
            "b_v": np.ascontiguousarray(b_qkv[2 * D + g * CH:
                                              2 * D + (g + 1) * CH]),
        })
    return in_maps


_PROGRAM = None


def _get_program():
    global _PROGRAM
    if _PROGRAM is None:
        _PROGRAM = build_core_program()
    return _PROGRAM


def run_sharded(nc, in_maps, **kw):
    """run_bass_kernel_spmd with retries: the first execution on a freshly
    attached device occasionally dies with NRT_EXEC_UNIT_UNRECOVERABLE."""
    last = None
    for _ in range(3):
        try:
            return run_bass_kernel_spmd(nc, in_maps,
                                        core_ids=list(range(N_CORES)), **kw)
        except Exception as e:  # noqa: BLE001
            last = e
    raise last


def kernel(x, w_qkv, b_qkv, w_proj, b_proj):
    x = np.asarray(x, dtype=np.float32)
    w_qkv = np.asarray(w_qkv, dtype=np.float32)
    b_qkv = np.asarray(b_qkv, dtype=np.float32)
    w_proj = np.asarray(w_proj, dtype=np.float32)
    b_proj = np.asarray(b_proj, dtype=np.float32)

    nc = _get_program()
    in_maps = shard_inputs(x, w_qkv, b_qkv, w_proj)
    res = run_sharded(nc, in_maps)

    B, S, D = x.shape
    out = np.empty((B, S, D), dtype=np.float32)
    for b in range(B):
        out[b] = res.results[2 * b]["out"] + res.results[2 * b + 1]["out"] + b_proj
    return out
